# revision 69
# baseline (speedup 1.0000x reference)
"""Trainium2 Bass kernel for nn_Nibbler_70755291234540 (gnn_message_passing).

q = concat(obs, relu(per-gvf tiny nets(gathered obs))) @ q_W.T

Strategy (8 NeuronCores, SPMD single program):
  - Shard the 4096 GVFs across cores (512/core); every core sees the full
    batch and produces partial Q strips; host sums the partials.
  - Host pre-transposes obs -> obsT (4096, 2048) in *fp8e4m3* in DRAM. The
    per-GVF input gather is a row gather out of obsT via dma_gather (GPSIMD
    SWDGE): 128 gathered rows = one group of 8 GVFs x 16 inputs, 2KB/row.
    fp8 halves the gather DMA (the baseline bottleneck) to 16MB/core.
  - Stage 1: per pair of groups, fp8 matmuls col-tiled onto PE column halves
    (cols 0-63 / 64-127) -> [128, 1024] f32 PSUM tiles; relu+f16 eviction on
    ACT/DVE uses all 128 lanes (the baseline's [64, x] tiles wasted half).
  - Q head in f16 for precision: per pair-tile a [128, 32-padded] lhsT of
    q_W columns, 4-way col-tiled into 4 PSUM strips (partitions 32s..32s+17)
    so 4 strips accumulate concurrently; host sums strips. The raw-obs part
    of the Q head is computed mid-pipeline from this core's 512-row slice of
    obsT kept in f16.
  - Q matmuls run one chunk behind the gathers (software pipelining) so the
    PE never waits on relu evictions.
"""

import sys
import types

import numpy as np
import ml_dtypes

# ---- problem constants (hardcoded; kernel.py must be self-contained) ----
B = 2048
OBS_DIM = 4096
N_GVFS = 4096
IPG = 16  # inputs per gvf
HPG = 8  # hidden per gvf
NA = 18  # actions
NAP = 32  # actions padded to a PE col-group
N_CORES = 8
GPC = N_GVFS // N_CORES  # 512 gvfs per core
N_GROUPS = GPC // 8  # 64 groups of 8 gvfs -> 128 gathered rows each
N_PAIRS = N_GROUPS // 2  # 32 pair-tiles of 128 feat rows
NB = 512  # matmul moving-operand chunk
CHUNK_GROUPS = 8  # gvf groups per dma_gather call (1024 rows, full batch)
N_CHUNKS = N_GROUPS // CHUNK_GROUPS  # 8
OWN_BLKS = (OBS_DIM // N_CORES) // 128  # 4 obs-feature blocks per core

F8NP = ml_dtypes.float8_e4m3


def _install_axon_profile_hook():
    """bass_utils trace=True under axon needs antenv.axon_hooks; shim it."""
    try:
        import antenv
    except ImportError:
        return
    if "antenv.axon_hooks" in sys.modules:
        return
    hooks = types.ModuleType("antenv.axon_hooks")
    hooks._hook = None

    def set_axon_ntff_profile_hook(h):
        hooks._hook = h

    def get_axon_ntff_profile_hook():
        return hooks._hook

    hooks.set_axon_ntff_profile_hook = set_axon_ntff_profile_hook
    hooks.get_axon_ntff_profile_hook = get_axon_ntff_profile_hook
    sys.modules["antenv.axon_hooks"] = hooks
    antenv.axon_hooks = hooks
    try:
        from trn_agent_boot.trn_boot import _ntff_profile_via_ctypes

        hook = _ntff_profile_via_ctypes("/opt/axon/libaxon_pjrt.so")
        if hook is not None:
            set_axon_ntff_profile_hook(hook)
    except Exception:
        pass


_install_axon_profile_hook()

import concourse.bacc as bacc
import concourse.bass as bass
import concourse.mybir as mybir
import concourse.tile as tile
from concourse.bass_utils import run_bass_kernel_spmd

F8 = mybir.dt.float8e4
F16 = mybir.dt.float16
F32 = mybir.dt.float32
I16 = mybir.dt.int16

_PROGRAM = None


def _build_program():
    nc = bacc.Bacc(
        None,
        target_bir_lowering=False,
        debug=False,
        num_devices=N_CORES,
        num_swdge_queues=4,
    )

    obst8 = nc.dram_tensor("obst8", [OBS_DIM, B], F8, kind="ExternalInput")
    obso = nc.dram_tensor("obso", [OWN_BLKS * 128, B], F16, kind="ExternalInput")
    wbd = nc.dram_tensor("wbd", [128, N_GROUPS * 64], F8, kind="ExternalInput")
    qwt = nc.dram_tensor("qwt", [128, N_PAIRS * NAP], F16, kind="ExternalInput")
    qwto = nc.dram_tensor("qwto", [128, OWN_BLKS * NAP], F16, kind="ExternalInput")
    gidx = nc.dram_tensor("gidx", [128, N_GROUPS * 8], I16, kind="ExternalInput")
    qp = nc.dram_tensor("qp", [128, B], F16, kind="ExternalOutput")

    with tile.TileContext(nc) as tc:
        with (
            tc.tile_pool(name="const", bufs=1) as const,
            tc.tile_pool(name="gbuf", bufs=4) as gbuf,
            tc.tile_pool(name="fbuf", bufs=3) as fbuf,
            tc.tile_pool(name="qout", bufs=1) as qout,
            tc.tile_pool(name="pre_ps", bufs=4, space="PSUM") as pre_ps,
            tc.tile_pool(name="qacc_ps", bufs=1, space="PSUM") as qacc_ps,
        ):
            gidx_sb = const.tile([128, N_GROUPS * 8], I16)
            wbd_sb = const.tile([128, N_GROUPS * 64], F8)
            qwt_sb = const.tile([128, N_PAIRS * NAP], F16)
            qwto_sb = const.tile([128, OWN_BLKS * NAP], F16)
            obso_sb = const.tile([128, OWN_BLKS, B], F16)
            warm_idx = const.tile([128, 1], I16)
            warm_out = const.tile([128, 1, B], F8)
            # gidx first: it gates the first dma_gather's descriptor gen
            nc.sync.dma_start(gidx_sb[:], gidx[:])
            nc.sync.dma_start(wbd_sb[:], wbd[:])

            qacc = qacc_ps.tile([128, B], F32)
            qsb = qout.tile([128, B], F16)

            # 16-idx dummy gather: absorbs the first dma_gather call's ~2.7us
            # cold-path setup (later calls cost ~74ns) while gidx still loads,
            # so the real chunk-0 transfers start earlier
            nc.vector.memset(warm_idx[:], 0)
            nc.gpsimd.dma_gather(
                out_ap=warm_out[:],
                in_ap=obst8[:],
                idxs_ap=warm_idx[:],
                num_idxs=16,
                num_idxs_reg=16,
                elem_size=B,
                    single_packet=False,
                queue_num=0,
            )

            # gathers: emitted up-front; gbuf slot reuse paces them. Chunk 0
            # is split into four 256-row sub-gathers (one per pair, one per
            # queue) so the first stage-1 matmuls can start as soon as the
            # first 0.5MB lands instead of waiting for the full 2.1MB.
            gt0 = []
            for sub in range(4):
                g = gbuf.tile([128, 2, B], F8, tag="gt0", bufs=4, name=f"gt0_{sub}")
                nc.gpsimd.dma_gather(
                    out_ap=g[:],
                    in_ap=obst8[:],
                    idxs_ap=gidx_sb[:, sub * 16 : (sub + 1) * 16],
                    num_idxs=256,
                    num_idxs_reg=256,
                    elem_size=B,
                    single_packet=False,
                    queue_num=sub,
                )
                gt0.append(g)
            gts = [gt0]
            for c in range(1, N_CHUNKS):
                gt = gbuf.tile([128, CHUNK_GROUPS, B], F8, tag="gt", name=f"gt{c}")
                nc.gpsimd.dma_gather(
                    out_ap=gt[:],
                    in_ap=obst8[:],
                    idxs_ap=gidx_sb[:, c * 64 : (c + 1) * 64],
                    num_idxs=CHUNK_GROUPS * 128,
                    num_idxs_reg=CHUNK_GROUPS * 128,
                    elem_size=B,
                    single_packet=False,
                    queue_num=c % 4,
                )
                gts.append(gt)

            # remaining consts behind the gather-critical ones; obso via the
            # ACT HWDGE ring so its dispatch parallels sync's
            nc.sync.dma_start(qwt_sb[:], qwt[:])
            nc.sync.dma_start(qwto_sb[:], qwto[:])
            for ob in range(OWN_BLKS):
                nc.scalar.dma_start(obso_sb[:, ob, :], obso[ob * 128 : (ob + 1) * 128, :])

            # PE warmup: garbage matmuls (cleared later by the start=True obs
            # matmuls) keep the PE HAM busy so real work runs at 2.4 GHz
            for w in range(32):
                nc.tensor.matmul(
                    qacc[0:64, 0:NB],
                    wbd_sb[:, 0:64],
                    wbd_sb[:, 0:NB],
                    start=True,
                    stop=True,
                    skip_group_check=True,
                )

            evict_n = 0
            feats_prev = [None] * 4
            NB2 = 2 * NB  # 1024-wide moving operands (fp8/f16 max)

            def emit_obs_q():
                # raw-obs part of the Q head: block s -> strip s; M=32
                # zero-padded so PSUM rows 18-31 of each strip are defined.
                # start=True: first real writer of every qacc element.
                for s in range(4):
                    lhsT = qwto_sb[:, s * NAP : (s + 1) * NAP]
                    for nb in range(B // NB):
                        nc.tensor.matmul(
                            qacc[32 * s : 32 * s + NAP, nb * NB : (nb + 1) * NB],
                            lhsT,
                            obso_sb[:, s, nb * NB : (nb + 1) * NB],
                            start=True,
                            stop=False,
                            tile_position=(0, 32 * s),
                            skip_group_check=True,
                        )

            def emit_q_pairs(c, first, stop):
                # Q-head matmuls for the 4 pairs of chunk c (strip-major:
                # strip j chains overlap on distinct PE col-groups)
                for j in range(4):
                    t2 = 4 * c + j
                    lhsT = qwt_sb[:, t2 * NAP : (t2 + 1) * NAP]
                    for nb in range(B // NB):
                        nc.tensor.matmul(
                            qacc[32 * j : 32 * j + NAP, nb * NB : (nb + 1) * NB],
                            lhsT,
                            feats_prev[j][:, nb * NB : (nb + 1) * NB],
                            start=first,
                            stop=stop,
                            tile_position=(0, 32 * j),
                            skip_group_check=True,
                        )

            feats_cur = [None] * 4
            for c in range(N_CHUNKS):
                for j in range(4):
                    t = 4 * c + j
                    ga, gb = 2 * t, 2 * t + 1
                    if c == 0:
                        rhs_a, rhs_b = gt0[j][:, 0, :], gt0[j][:, 1, :]
                    else:
                        gt = gts[c]
                        rhs_a, rhs_b = gt[:, 2 * j, :], gt[:, 2 * j + 1, :]
                    feat = fbuf.tile([128, B], F16, tag=f"feat{j}", name=f"feat_{t}")
                    feats_cur[j] = feat
                    for blk in range(4):
                        pre = pre_ps.tile(
                            [128, NB], F32, tag="pre", name=f"pre_{t}_{blk}"
                        )
                        # group A on PE cols 0-63, group B on cols 64-127:
                        # disjoint col-groups -> the two MMs overlap. 4 PSUM
                        # slots of one bank each keep evictions off the
                        # critical path (slot free != eviction latency).
                        col = blk * NB
                        nc.tensor.matmul(
                            pre[0:64, :],
                            wbd_sb[:, ga * 64 : (ga + 1) * 64],
                            rhs_a[:, col : col + NB],
                            start=True,
                            stop=True,
                        )
                        nc.tensor.matmul(
                            pre[64:128, :],
                            wbd_sb[:, gb * 64 : (gb + 1) * 64],
                            rhs_b[:, col : col + NB],
                            start=True,
                            stop=True,
                        )
                        dst = feat[:, col : col + NB]
                        if evict_n % 2 == 0:
                            nc.scalar.activation(
                                dst, pre[:], mybir.ActivationFunctionType.Relu
                            )
                        else:
                            nc.vector.tensor_scalar_max(dst, pre[:], 0.0)
                        evict_n += 1
                # Q head for the previous chunk's pairs (software pipeline:
                # their relu evictions finished during this chunk's stage 1).
                # Chunk 0's slot computes the raw-obs Q part instead.
                if c >= 1:
                    emit_q_pairs(c - 1, first=False, stop=False)
                else:
                    emit_obs_q()
                feats_prev, feats_cur = feats_cur, feats_prev

            emit_q_pairs(N_CHUNKS - 1, first=False, stop=True)

            nc.scalar.activation(
                qsb[:, 0 : B // 2],
                qacc[:, 0 : B // 2],
                mybir.ActivationFunctionType.Copy,
            )
            nc.vector.tensor_copy(qsb[:, B // 2 : B], qacc[:, B // 2 : B])
            nc.sync.dma_start(qp[:, 0 : B // 2], qsb[:, 0 : B // 2])
            nc.scalar.dma_start(qp[:, B // 2 : B], qsb[:, B // 2 : B])

    nc.finalize()
    return nc


def _get_program():
    global _PROGRAM
    if _PROGRAM is None:
        _PROGRAM = _build_program()
    return _PROGRAM


def _stage_inputs(observation, gvf_W, q_W, gvf_input_idxs):
    """Host-side sharding/layout. Returns in_maps (list of dicts, one per core)."""
    obs = np.asarray(observation, dtype=np.float32)
    gw = np.asarray(gvf_W, dtype=np.float32)
    qw = np.asarray(q_W, dtype=np.float32)
    idx = np.asarray(gvf_input_idxs).astype(np.int64)

    obst16 = np.ascontiguousarray(obs.T.astype(np.float16))  # (OBS_DIM, B)
    obst8 = np.ascontiguousarray(obs.T.astype(F8NP))  # (OBS_DIM, B) fp8

    in_maps = []
    for c in range(N_CORES):
        g0 = c * GPC

        # gather index plan: j = k*128 + p ; p = 16*a + i
        # idx_flat[j] = idx[g0 + 8k + a, i]
        k = np.arange(N_GROUPS)[:, None, None]  # group
        a = np.arange(8)[None, :, None]  # gvf within group
        i = np.arange(IPG)[None, None, :]  # input slot
        idx_flat = idx[g0 + 8 * k + a, i].reshape(N_GROUPS * 128)  # (8192,)
        # wrapped for dma_gather: per call of n idxs, wrapped[p, s] = flat[s*16+p%16]
        # chunk 0 is issued as four 256-idx sub-calls, chunks 1-7 as 1024 each
        per_call = CHUNK_GROUPS * 128
        call_bounds = [(sub * 256, (sub + 1) * 256) for sub in range(4)] + [
            (chunk * per_call, (chunk + 1) * per_call) for chunk in range(1, N_CHUNKS)
        ]
        gidx_h = np.zeros((128, N_GROUPS * 8), dtype=np.int16)
        colp = 0
        for lo, hi in call_bounds:
            fl = idx_flat[lo:hi]
            ncols = (hi - lo) // 16
            wr = fl.reshape(ncols, 16).T  # (16, S): wr[p, s] = fl[s*16+p]
            gidx_h[:, colp : colp + ncols] = np.tile(wr, (8, 1))
            colp += ncols

        # block-diagonal gvf weights: wbd[p, 64k + 8a + h] = gw[g0+8k+a, h, i]
        # with p = 16a + i
        wbd_h = np.zeros((128, N_GROUPS * 64), dtype=F8NP)
        kk = np.arange(N_GROUPS)[:, None, None, None]
        aa = np.arange(8)[None, :, None, None]
        hh = np.arange(HPG)[None, None, :, None]
        ii = np.arange(IPG)[None, None, None, :]
        vals = gw[g0 + 8 * kk + aa, hh, ii]  # (64, 8, 8, 16)
        p_idx = (16 * aa + ii).reshape(1, 8, 1, IPG)
        m_idx = (64 * kk + 8 * aa + hh).reshape(N_GROUPS, 8, HPG, 1)
        pf = np.broadcast_to(p_idx, vals.shape).reshape(-1)
        mf = np.broadcast_to(m_idx, vals.shape).reshape(-1)
        wbd_h[pf, mf] = vals.astype(F8NP).reshape(-1)

        # q-head weights for gvf features: pair tile P covers feat rows
        # pp in [0,128): k = 2P + pp//64, m = pp%64, gvf = g0+8k+m//8, h = m%8
        # padded to NAP=32 action slots per pair (rows 18-31 zero)
        P = np.arange(N_PAIRS)[None, :]
        pp = np.arange(128)[:, None]
        kq = 2 * P + pp // 64
        m = pp % 64
        col = OBS_DIM + (g0 + 8 * kq + m // 8) * HPG + (m % 8)  # (128, 32)
        qwt_h = np.zeros((128, N_PAIRS, NAP), dtype=np.float16)
        qwt_h[:, :, :NA] = qw[:, col].transpose(1, 2, 0).astype(np.float16)
        qwt_h = qwt_h.reshape(128, N_PAIRS * NAP)

        # q-head weights for this core's raw-obs blocks, padded to NAP
        f0 = c * (OBS_DIM // N_CORES)
        colo = f0 + np.arange(OWN_BLKS)[None, :] * 128 + np.arange(128)[:, None]
        qwto_h = np.zeros((128, OWN_BLKS, NAP), dtype=np.float16)
        qwto_h[:, :, :NA] = qw[:, colo].transpose(1, 2, 0).astype(np.float16)
        qwto_h = qwto_h.reshape(128, OWN_BLKS * NAP)

        obso_h = np.ascontiguousarray(obst16[f0 : f0 + OWN_BLKS * 128, :])

        in_maps.append(
            {
                "obst8": obst8,
                "obso": obso_h,
                "wbd": np.ascontiguousarray(wbd_h),
                "qwt": np.ascontiguousarray(qwt_h),
                "qwto": np.ascontiguousarray(qwto_h),
                "gidx": gidx_h,
            }
        )
    return in_maps


def kernel(observation, gvf_W, q_W, gvf_input_idxs, _trace=False):
    nc = _get_program()
    in_maps = _stage_inputs(observation, gvf_W, q_W, gvf_input_idxs)
    res = run_bass_kernel_spmd(nc, in_maps, list(range(N_CORES)), trace=_trace)
    q = np.zeros((NA, B), dtype=np.float32)
    for c in range(N_CORES):
        qpc = res.results[c]["qp"].astype(np.float32)  # [128, B]
        for s in range(4):
            q += qpc[32 * s : 32 * s + NA, :]
    out = np.ascontiguousarray(q.T, dtype=np.float32)
    if _trace:
        kernel.last_exec_time_ns = res.exec_time_ns
    return out


# revision 70
# speedup vs baseline: 1.2037x; 1.2037x over previous
"""Trainium2 Bass kernel for nn_Nibbler_70755291234540 (gnn_message_passing).

q = concat(obs, relu(per-gvf tiny nets(gathered obs))) @ q_W.T

Strategy (8 NeuronCores, SPMD single program):
  - Shard the 4096 GVFs across cores (512/core); every core sees the full
    batch and produces partial Q strips; host sums the partials.
  - Host pre-transposes obs -> obsT (4096, 2048) in *fp8e4m3* in DRAM. The
    per-GVF input gather is a row gather out of obsT via dma_gather (GPSIMD
    SWDGE): 128 gathered rows = one group of 8 GVFs x 16 inputs, 2KB/row.
    fp8 halves the gather DMA (the baseline bottleneck) to 16MB/core.
  - Stage 1: per pair of groups, fp8 matmuls col-tiled onto PE column halves
    (cols 0-63 / 64-127) -> [128, 1024] f32 PSUM tiles; relu+f16 eviction on
    ACT/DVE uses all 128 lanes (the baseline's [64, x] tiles wasted half).
  - Q head in f16 for precision: per pair-tile a [128, 32-padded] lhsT of
    q_W columns, 4-way col-tiled into 4 PSUM strips (partitions 32s..32s+17)
    so 4 strips accumulate concurrently; host sums strips. The raw-obs part
    of the Q head is computed mid-pipeline from this core's 512-row slice of
    obsT kept in f16.
  - Q matmuls run one chunk behind the gathers (software pipelining) so the
    PE never waits on relu evictions.
"""

import sys
import types

import numpy as np
import ml_dtypes

# ---- problem constants (hardcoded; kernel.py must be self-contained) ----
B = 2048
OBS_DIM = 4096
N_GVFS = 4096
IPG = 16  # inputs per gvf
HPG = 8  # hidden per gvf
NA = 18  # actions
NAP = 32  # actions padded to a PE col-group
N_CORES = 8
GPC = N_GVFS // N_CORES  # 512 gvfs per core
N_GROUPS = GPC // 8  # 64 groups of 8 gvfs -> 128 gathered rows each
N_PAIRS = N_GROUPS // 2  # 32 pair-tiles of 128 feat rows
NB = 512  # matmul moving-operand chunk
CHUNK_GROUPS = 8  # gvf groups per dma_gather call (1024 rows, full batch)
N_CHUNKS = N_GROUPS // CHUNK_GROUPS  # 8
OWN_BLKS = (OBS_DIM // N_CORES) // 128  # 4 obs-feature blocks per core

F8NP = ml_dtypes.float8_e4m3


def _install_axon_profile_hook():
    """bass_utils trace=True under axon needs antenv.axon_hooks; shim it."""
    try:
        import antenv
    except ImportError:
        return
    if "antenv.axon_hooks" in sys.modules:
        return
    hooks = types.ModuleType("antenv.axon_hooks")
    hooks._hook = None

    def set_axon_ntff_profile_hook(h):
        hooks._hook = h

    def get_axon_ntff_profile_hook():
        return hooks._hook

    hooks.set_axon_ntff_profile_hook = set_axon_ntff_profile_hook
    hooks.get_axon_ntff_profile_hook = get_axon_ntff_profile_hook
    sys.modules["antenv.axon_hooks"] = hooks
    antenv.axon_hooks = hooks
    try:
        from trn_agent_boot.trn_boot import _ntff_profile_via_ctypes

        hook = _ntff_profile_via_ctypes("/opt/axon/libaxon_pjrt.so")
        if hook is not None:
            set_axon_ntff_profile_hook(hook)
    except Exception:
        pass


_install_axon_profile_hook()

import concourse.bacc as bacc
import concourse.bass as bass
import concourse.mybir as mybir
import concourse.tile as tile
from concourse.bass_utils import run_bass_kernel_spmd

F8 = mybir.dt.float8e4
F16 = mybir.dt.float16
F32 = mybir.dt.float32
I16 = mybir.dt.int16

_PROGRAM = None


def _build_program():
    nc = bacc.Bacc(
        None,
        target_bir_lowering=False,
        debug=False,
        num_devices=N_CORES,
        num_swdge_queues=4,
    )

    obst8 = nc.dram_tensor("obst8", [OBS_DIM, B], F8, kind="ExternalInput")
    obso = nc.dram_tensor("obso", [OWN_BLKS * 128, B], F16, kind="ExternalInput")
    wbd = nc.dram_tensor("wbd", [128, N_GROUPS * 64], F8, kind="ExternalInput")
    qwt = nc.dram_tensor("qwt", [128, N_PAIRS * NAP], F16, kind="ExternalInput")
    qwto = nc.dram_tensor("qwto", [128, OWN_BLKS * NAP], F16, kind="ExternalInput")
    gidx = nc.dram_tensor("gidx", [128, N_GROUPS * 8], I16, kind="ExternalInput")
    qp = nc.dram_tensor("qp", [128, B], F16, kind="ExternalOutput")

    with tile.TileContext(nc) as tc:
        with (
            tc.tile_pool(name="const", bufs=1) as const,
            tc.tile_pool(name="gbuf", bufs=4) as gbuf,
            tc.tile_pool(name="fbuf", bufs=3) as fbuf,
            tc.tile_pool(name="qout", bufs=1) as qout,
            tc.tile_pool(name="pre_ps", bufs=4, space="PSUM") as pre_ps,
            tc.tile_pool(name="qacc_ps", bufs=1, space="PSUM") as qacc_ps,
        ):
            gidx_sb = const.tile([128, N_GROUPS * 8], I16)
            wbd_sb = const.tile([128, N_GROUPS * 64], F8)
            qwt_sb = const.tile([128, N_PAIRS * NAP], F16)
            qwto_sb = const.tile([128, OWN_BLKS * NAP], F16)
            obso_sb = const.tile([128, OWN_BLKS, B], F16)
            warm_idx = const.tile([128, 1], I16)
            warm_out = const.tile([128, 1, B], F8)
            # gidx first: it gates the first dma_gather's descriptor gen
            nc.sync.dma_start(gidx_sb[:], gidx[:])
            nc.sync.dma_start(wbd_sb[:], wbd[:])

            qacc = qacc_ps.tile([128, B], F32)
            qsb = qout.tile([128, B], F16)

            # 16-idx dummy gather: absorbs the first dma_gather call's ~2.7us
            # cold-path setup (later calls cost ~74ns) while gidx still loads,
            # so the real chunk-0 transfers start earlier
            nc.vector.memset(warm_idx[:], 0)
            nc.gpsimd.dma_gather(
                out_ap=warm_out[:],
                in_ap=obst8[:],
                idxs_ap=warm_idx[:],
                num_idxs=16,
                num_idxs_reg=16,
                elem_size=B,
                queue_num=0,
            )

            # gathers: emitted up-front; gbuf slot reuse paces them. Chunk 0
            # is split into four 256-row sub-gathers (one per pair, one per
            # queue) so the first stage-1 matmuls can start as soon as the
            # first 0.5MB lands instead of waiting for the full 2.1MB.
            gt0 = []
            for sub in range(4):
                g = gbuf.tile([128, 2, B], F8, tag="gt0", bufs=4, name=f"gt0_{sub}")
                nc.gpsimd.dma_gather(
                    out_ap=g[:],
                    in_ap=obst8[:],
                    idxs_ap=gidx_sb[:, sub * 16 : (sub + 1) * 16],
                    num_idxs=256,
                    num_idxs_reg=256,
                    elem_size=B,
                    queue_num=sub,
                )
                gt0.append(g)
            gts = [gt0]
            for c in range(1, N_CHUNKS):
                gt = gbuf.tile([128, CHUNK_GROUPS, B], F8, tag="gt", name=f"gt{c}")
                nc.gpsimd.dma_gather(
                    out_ap=gt[:],
                    in_ap=obst8[:],
                    idxs_ap=gidx_sb[:, c * 64 : (c + 1) * 64],
                    num_idxs=CHUNK_GROUPS * 128,
                    num_idxs_reg=CHUNK_GROUPS * 128,
                    elem_size=B,
                    queue_num=c % 4,
                )
                gts.append(gt)

            # remaining consts behind the gather-critical ones; obso via the
            # ACT HWDGE ring so its dispatch parallels sync's
            nc.sync.dma_start(qwt_sb[:], qwt[:])
            nc.sync.dma_start(qwto_sb[:], qwto[:])
            for ob in range(OWN_BLKS):
                nc.scalar.dma_start(obso_sb[:, ob, :], obso[ob * 128 : (ob + 1) * 128, :])

            # PE warmup: garbage matmuls (cleared later by the start=True obs
            # matmuls) keep the PE HAM busy so real work runs at 2.4 GHz
            for w in range(32):
                nc.tensor.matmul(
                    qacc[0:64, 0:NB],
                    wbd_sb[:, 0:64],
                    wbd_sb[:, 0:NB],
                    start=True,
                    stop=True,
                    skip_group_check=True,
                )

            evict_n = 0
            feats_prev = [None] * 4
            NB2 = 2 * NB  # 1024-wide moving operands (fp8/f16 max)

            def emit_obs_q():
                # raw-obs part of the Q head: block s -> strip s; M=32
                # zero-padded so PSUM rows 18-31 of each strip are defined.
                # start=True: first real writer of every qacc element.
                for s in range(4):
                    lhsT = qwto_sb[:, s * NAP : (s + 1) * NAP]
                    for nb in range(B // NB):
                        nc.tensor.matmul(
                            qacc[32 * s : 32 * s + NAP, nb * NB : (nb + 1) * NB],
                            lhsT,
                            obso_sb[:, s, nb * NB : (nb + 1) * NB],
                            start=True,
                            stop=False,
                            tile_position=(0, 32 * s),
                            skip_group_check=True,
                        )

            def emit_q_pairs(c, first, stop):
                # Q-head matmuls for the 4 pairs of chunk c (strip-major:
                # strip j chains overlap on distinct PE col-groups)
                for j in range(4):
                    t2 = 4 * c + j
                    lhsT = qwt_sb[:, t2 * NAP : (t2 + 1) * NAP]
                    for nb in range(B // NB):
                        nc.tensor.matmul(
                            qacc[32 * j : 32 * j + NAP, nb * NB : (nb + 1) * NB],
                            lhsT,
                            feats_prev[j][:, nb * NB : (nb + 1) * NB],
                            start=first,
                            stop=stop,
                            tile_position=(0, 32 * j),
                            skip_group_check=True,
                        )

            feats_cur = [None] * 4
            for c in range(N_CHUNKS):
                for j in range(4):
                    t = 4 * c + j
                    ga, gb = 2 * t, 2 * t + 1
                    if c == 0:
                        rhs_a, rhs_b = gt0[j][:, 0, :], gt0[j][:, 1, :]
                    else:
                        gt = gts[c]
                        rhs_a, rhs_b = gt[:, 2 * j, :], gt[:, 2 * j + 1, :]
                    feat = fbuf.tile([128, B], F16, tag=f"feat{j}", name=f"feat_{t}")
                    feats_cur[j] = feat
                    for blk in range(4):
                        pre = pre_ps.tile(
                            [128, NB], F32, tag="pre", name=f"pre_{t}_{blk}"
                        )
                        # group A on PE cols 0-63, group B on cols 64-127:
                        # disjoint col-groups -> the two MMs overlap. 4 PSUM
                        # slots of one bank each keep evictions off the
                        # critical path (slot free != eviction latency).
                        col = blk * NB
                        nc.tensor.matmul(
                            pre[0:64, :],
                            wbd_sb[:, ga * 64 : (ga + 1) * 64],
                            rhs_a[:, col : col + NB],
                            start=True,
                            stop=True,
                        )
                        nc.tensor.matmul(
                            pre[64:128, :],
                            wbd_sb[:, gb * 64 : (gb + 1) * 64],
                            rhs_b[:, col : col + NB],
                            start=True,
                            stop=True,
                        )
                        dst = feat[:, col : col + NB]
                        if evict_n % 2 == 0:
                            nc.scalar.activation(
                                dst, pre[:], mybir.ActivationFunctionType.Relu
                            )
                        else:
                            nc.vector.tensor_scalar_max(dst, pre[:], 0.0)
                        evict_n += 1
                # Q head for the previous chunk's pairs (software pipeline:
                # their relu evictions finished during this chunk's stage 1).
                # Chunk 0's slot computes the raw-obs Q part instead.
                if c >= 1:
                    emit_q_pairs(c - 1, first=False, stop=False)
                else:
                    emit_obs_q()
                feats_prev, feats_cur = feats_cur, feats_prev

            emit_q_pairs(N_CHUNKS - 1, first=False, stop=True)

            nc.scalar.activation(
                qsb[:, 0 : B // 2],
                qacc[:, 0 : B // 2],
                mybir.ActivationFunctionType.Copy,
            )
            nc.vector.tensor_copy(qsb[:, B // 2 : B], qacc[:, B // 2 : B])
            nc.sync.dma_start(qp[:, 0 : B // 2], qsb[:, 0 : B // 2])
            nc.scalar.dma_start(qp[:, B // 2 : B], qsb[:, B // 2 : B])

    nc.finalize()
    return nc


def _get_program():
    global _PROGRAM
    if _PROGRAM is None:
        _PROGRAM = _build_program()
    return _PROGRAM


def _stage_inputs(observation, gvf_W, q_W, gvf_input_idxs):
    """Host-side sharding/layout. Returns in_maps (list of dicts, one per core)."""
    obs = np.asarray(observation, dtype=np.float32)
    gw = np.asarray(gvf_W, dtype=np.float32)
    qw = np.asarray(q_W, dtype=np.float32)
    idx = np.asarray(gvf_input_idxs).astype(np.int64)

    obst16 = np.ascontiguousarray(obs.T.astype(np.float16))  # (OBS_DIM, B)
    obst8 = np.ascontiguousarray(obs.T.astype(F8NP))  # (OBS_DIM, B) fp8

    in_maps = []
    for c in range(N_CORES):
        g0 = c * GPC

        # gather index plan: j = k*128 + p ; p = 16*a + i
        # idx_flat[j] = idx[g0 + 8k + a, i]
        k = np.arange(N_GROUPS)[:, None, None]  # group
        a = np.arange(8)[None, :, None]  # gvf within group
        i = np.arange(IPG)[None, None, :]  # input slot
        idx_flat = idx[g0 + 8 * k + a, i].reshape(N_GROUPS * 128)  # (8192,)
        # wrapped for dma_gather: per call of n idxs, wrapped[p, s] = flat[s*16+p%16]
        # chunk 0 is issued as four 256-idx sub-calls, chunks 1-7 as 1024 each
        per_call = CHUNK_GROUPS * 128
        call_bounds = [(sub * 256, (sub + 1) * 256) for sub in range(4)] + [
            (chunk * per_call, (chunk + 1) * per_call) for chunk in range(1, N_CHUNKS)
        ]
        gidx_h = np.zeros((128, N_GROUPS * 8), dtype=np.int16)
        colp = 0
        for lo, hi in call_bounds:
            fl = idx_flat[lo:hi]
            ncols = (hi - lo) // 16
            wr = fl.reshape(ncols, 16).T  # (16, S): wr[p, s] = fl[s*16+p]
            gidx_h[:, colp : colp + ncols] = np.tile(wr, (8, 1))
            colp += ncols

        # block-diagonal gvf weights: wbd[p, 64k + 8a + h] = gw[g0+8k+a, h, i]
        # with p = 16a + i
        wbd_h = np.zeros((128, N_GROUPS * 64), dtype=F8NP)
        kk = np.arange(N_GROUPS)[:, None, None, None]
        aa = np.arange(8)[None, :, None, None]
        hh = np.arange(HPG)[None, None, :, None]
        ii = np.arange(IPG)[None, None, None, :]
        vals = gw[g0 + 8 * kk + aa, hh, ii]  # (64, 8, 8, 16)
        p_idx = (16 * aa + ii).reshape(1, 8, 1, IPG)
        m_idx = (64 * kk + 8 * aa + hh).reshape(N_GROUPS, 8, HPG, 1)
        pf = np.broadcast_to(p_idx, vals.shape).reshape(-1)
        mf = np.broadcast_to(m_idx, vals.shape).reshape(-1)
        wbd_h[pf, mf] = vals.astype(F8NP).reshape(-1)

        # q-head weights for gvf features: pair tile P covers feat rows
        # pp in [0,128): k = 2P + pp//64, m = pp%64, gvf = g0+8k+m//8, h = m%8
        # padded to NAP=32 action slots per pair (rows 18-31 zero)
        P = np.arange(N_PAIRS)[None, :]
        pp = np.arange(128)[:, None]
        kq = 2 * P + pp // 64
        m = pp % 64
        col = OBS_DIM + (g0 + 8 * kq + m // 8) * HPG + (m % 8)  # (128, 32)
        qwt_h = np.zeros((128, N_PAIRS, NAP), dtype=np.float16)
        qwt_h[:, :, :NA] = qw[:, col].transpose(1, 2, 0).astype(np.float16)
        qwt_h = qwt_h.reshape(128, N_PAIRS * NAP)

        # q-head weights for this core's raw-obs blocks, padded to NAP
        f0 = c * (OBS_DIM // N_CORES)
        colo = f0 + np.arange(OWN_BLKS)[None, :] * 128 + np.arange(128)[:, None]
        qwto_h = np.zeros((128, OWN_BLKS, NAP), dtype=np.float16)
        qwto_h[:, :, :NA] = qw[:, colo].transpose(1, 2, 0).astype(np.float16)
        qwto_h = qwto_h.reshape(128, OWN_BLKS * NAP)

        obso_h = np.ascontiguousarray(obst16[f0 : f0 + OWN_BLKS * 128, :])

        in_maps.append(
            {
                "obst8": obst8,
                "obso": obso_h,
                "wbd": np.ascontiguousarray(wbd_h),
                "qwt": np.ascontiguousarray(qwt_h),
                "qwto": np.ascontiguousarray(qwto_h),
                "gidx": gidx_h,
            }
        )
    return in_maps


def kernel(observation, gvf_W, q_W, gvf_input_idxs, _trace=False):
    nc = _get_program()
    in_maps = _stage_inputs(observation, gvf_W, q_W, gvf_input_idxs)
    res = run_bass_kernel_spmd(nc, in_maps, list(range(N_CORES)), trace=_trace)
    q = np.zeros((NA, B), dtype=np.float32)
    for c in range(N_CORES):
        qpc = res.results[c]["qp"].astype(np.float32)  # [128, B]
        for s in range(4):
            q += qpc[32 * s : 32 * s + NA, :]
    out = np.ascontiguousarray(q.T, dtype=np.float32)
    if _trace:
        kernel.last_exec_time_ns = res.exec_time_ns
    return out


# revision 72
# speedup vs baseline: 1.2390x; 1.0294x over previous
"""Trainium2 Bass kernel for nn_Nibbler_70755291234540 (gnn_message_passing).

q = concat(obs, relu(per-gvf tiny nets(gathered obs))) @ q_W.T

Strategy (8 NeuronCores, SPMD single program):
  - Shard the 4096 GVFs across cores (512/core); every core sees the full
    batch and produces partial Q strips; host sums the partials.
  - Host pre-transposes obs -> obsT (4096, 2048) in *fp8e4m3* in DRAM. The
    per-GVF input gather is a row gather out of obsT via dma_gather (GPSIMD
    SWDGE): 128 gathered rows = one group of 8 GVFs x 16 inputs, 2KB/row.
    fp8 halves the gather DMA (the baseline bottleneck) to 16MB/core.
  - Stage 1: per pair of groups, fp8 matmuls col-tiled onto PE column halves
    (cols 0-63 / 64-127) -> [128, 1024] f32 PSUM tiles; relu+f16 eviction on
    ACT/DVE uses all 128 lanes (the baseline's [64, x] tiles wasted half).
  - Q head in f16 for precision: per pair-tile a [128, 32-padded] lhsT of
    q_W columns, 4-way col-tiled into 4 PSUM strips (partitions 32s..32s+17)
    so 4 strips accumulate concurrently; host sums strips. The raw-obs part
    of the Q head is computed mid-pipeline from this core's 512-row slice of
    obsT kept in f16.
  - Q matmuls run one chunk behind the gathers (software pipelining) so the
    PE never waits on relu evictions.
"""

import sys
import types

import numpy as np
import ml_dtypes

# ---- problem constants (hardcoded; kernel.py must be self-contained) ----
B = 2048
OBS_DIM = 4096
N_GVFS = 4096
IPG = 16  # inputs per gvf
HPG = 8  # hidden per gvf
NA = 18  # actions
NAP = 32  # actions padded to a PE col-group
N_CORES = 8
GPC = N_GVFS // N_CORES  # 512 gvfs per core
N_GROUPS = GPC // 8  # 64 groups of 8 gvfs -> 128 gathered rows each
N_PAIRS = N_GROUPS // 2  # 32 pair-tiles of 128 feat rows
NB = 512  # matmul moving-operand chunk
CHUNK_GROUPS = 8  # gvf groups per dma_gather call (1024 rows, full batch)
N_CHUNKS = N_GROUPS // CHUNK_GROUPS  # 8
OWN_BLKS = (OBS_DIM // N_CORES) // 128  # 4 obs-feature blocks per core

F8NP = ml_dtypes.float8_e4m3


def _install_axon_profile_hook():
    """bass_utils trace=True under axon needs antenv.axon_hooks; shim it."""
    try:
        import antenv
    except ImportError:
        return
    if "antenv.axon_hooks" in sys.modules:
        return
    hooks = types.ModuleType("antenv.axon_hooks")
    hooks._hook = None

    def set_axon_ntff_profile_hook(h):
        hooks._hook = h

    def get_axon_ntff_profile_hook():
        return hooks._hook

    hooks.set_axon_ntff_profile_hook = set_axon_ntff_profile_hook
    hooks.get_axon_ntff_profile_hook = get_axon_ntff_profile_hook
    sys.modules["antenv.axon_hooks"] = hooks
    antenv.axon_hooks = hooks
    try:
        from trn_agent_boot.trn_boot import _ntff_profile_via_ctypes

        hook = _ntff_profile_via_ctypes("/opt/axon/libaxon_pjrt.so")
        if hook is not None:
            set_axon_ntff_profile_hook(hook)
    except Exception:
        pass


_install_axon_profile_hook()

import concourse.bacc as bacc
import concourse.bass as bass
import concourse.mybir as mybir
import concourse.tile as tile
from concourse.bass_utils import run_bass_kernel_spmd

F8 = mybir.dt.float8e4
F16 = mybir.dt.float16
F32 = mybir.dt.float32
I16 = mybir.dt.int16

_PROGRAM = None


def _build_program():
    nc = bacc.Bacc(
        None,
        target_bir_lowering=False,
        debug=False,
        num_devices=N_CORES,
        num_swdge_queues=4,
    )

    obst8 = nc.dram_tensor("obst8", [OBS_DIM, B], F8, kind="ExternalInput")
    obso = nc.dram_tensor("obso", [OWN_BLKS * 128, B], F16, kind="ExternalInput")
    wbd = nc.dram_tensor("wbd", [128, N_GROUPS * 64], F8, kind="ExternalInput")
    qwt = nc.dram_tensor("qwt", [128, N_PAIRS * NAP], F16, kind="ExternalInput")
    qwto = nc.dram_tensor("qwto", [128, OWN_BLKS * NAP], F16, kind="ExternalInput")
    gidx = nc.dram_tensor("gidx", [128, N_GROUPS * 8], I16, kind="ExternalInput")
    qp = nc.dram_tensor("qp", [128, B], F16, kind="ExternalOutput")

    with tile.TileContext(nc) as tc:
        with (
            tc.tile_pool(name="const", bufs=1) as const,
            tc.tile_pool(name="gbuf", bufs=4) as gbuf,
            tc.tile_pool(name="fbuf", bufs=3) as fbuf,
            tc.tile_pool(name="qout", bufs=1) as qout,
            tc.tile_pool(name="pre_ps", bufs=4, space="PSUM") as pre_ps,
            tc.tile_pool(name="qacc_ps", bufs=1, space="PSUM") as qacc_ps,
        ):
            gidx_sb = const.tile([128, N_GROUPS * 8], I16)
            wbd_sb = const.tile([128, N_GROUPS * 64], F8)
            qwt_sb = const.tile([128, N_PAIRS * NAP], F16)
            qwto_sb = const.tile([128, OWN_BLKS * NAP], F16)
            obso_sb = const.tile([128, OWN_BLKS, B], F16)
            warm_idx = const.tile([128, 1], I16)
            warm_out = const.tile([128, 1, B], F8)
            # gidx first: it gates the first dma_gather's descriptor gen
            nc.sync.dma_start(gidx_sb[:], gidx[:])
            nc.sync.dma_start(wbd_sb[:], wbd[:])

            qacc = qacc_ps.tile([128, B], F32)
            qsb = qout.tile([128, B], F16)

            # 16-idx dummy gather: absorbs the first dma_gather call's ~2.7us
            # cold-path setup (later calls cost ~74ns) while gidx still loads,
            # so the real chunk-0 transfers start earlier
            nc.vector.memset(warm_idx[:], 0)
            nc.gpsimd.dma_gather(
                out_ap=warm_out[:],
                in_ap=obst8[:],
                idxs_ap=warm_idx[:],
                num_idxs=16,
                num_idxs_reg=16,
                elem_size=B,
                queue_num=0,
            )

            # gathers: emitted up-front; gbuf slot reuse paces them. Chunk 0
            # is split into four 256-row sub-gathers (one per pair, one per
            # queue) so the first stage-1 matmuls can start as soon as the
            # first 0.5MB lands instead of waiting for the full 2.1MB.
            gt0 = []
            for sub in range(4):
                g = gbuf.tile([128, 2, B], F8, tag="gt0", bufs=4, name=f"gt0_{sub}")
                nc.gpsimd.dma_gather(
                    out_ap=g[:],
                    in_ap=obst8[:],
                    idxs_ap=gidx_sb[:, sub * 16 : (sub + 1) * 16],
                    num_idxs=256,
                    num_idxs_reg=256,
                    elem_size=B,
                    queue_num=sub,
                )
                gt0.append(g)
            gts = [gt0]
            for c in range(1, N_CHUNKS):
                gt = gbuf.tile([128, CHUNK_GROUPS, B], F8, tag="gt", name=f"gt{c}")
                nc.gpsimd.dma_gather(
                    out_ap=gt[:],
                    in_ap=obst8[:],
                    idxs_ap=gidx_sb[:, c * 64 : (c + 1) * 64],
                    num_idxs=CHUNK_GROUPS * 128,
                    num_idxs_reg=CHUNK_GROUPS * 128,
                    elem_size=B,
                    queue_num=c % 4,
                )
                gts.append(gt)

            # remaining consts behind the gather-critical ones; obso via the
            # ACT HWDGE ring so its dispatch parallels sync's
            nc.sync.dma_start(qwt_sb[:], qwt[:])
            nc.sync.dma_start(qwto_sb[:], qwto[:])
            for ob in range(OWN_BLKS):
                nc.scalar.dma_start(obso_sb[:, ob, :], obso[ob * 128 : (ob + 1) * 128, :])

            # PE warmup: garbage matmuls (cleared later by the start=True obs
            # matmuls) keep the PE HAM busy so real work runs at 2.4 GHz
            for w in range(32):
                nc.tensor.matmul(
                    qacc[0:64, 0:NB],
                    wbd_sb[:, 0:64],
                    wbd_sb[:, 0:NB],
                    start=True,
                    stop=True,
                    skip_group_check=True,
                )

            evict_n = 0
            feats_prev = [None] * 4
            NB2 = 2 * NB  # 1024-wide moving operands (fp8/f16 max)

            def emit_obs_q():
                # raw-obs part of the Q head: block s -> strip s; M=32
                # zero-padded so PSUM rows 18-31 of each strip are defined.
                # start=True: first real writer of every qacc element.
                for s in range(4):
                    lhsT = qwto_sb[:, s * NAP : (s + 1) * NAP]
                    for nb in range(B // NB):
                        nc.tensor.matmul(
                            qacc[32 * s : 32 * s + NAP, nb * NB : (nb + 1) * NB],
                            lhsT,
                            obso_sb[:, s, nb * NB : (nb + 1) * NB],
                            start=True,
                            stop=False,
                            tile_position=(0, 32 * s),
                            skip_group_check=True,
                        )

            def emit_q_pairs(c, first, stop):
                # Q-head matmuls for the 4 pairs of chunk c (strip-major:
                # strip j chains overlap on distinct PE col-groups)
                for j in range(4):
                    t2 = 4 * c + j
                    lhsT = qwt_sb[:, t2 * NAP : (t2 + 1) * NAP]
                    for nb in range(B // NB):
                        nc.tensor.matmul(
                            qacc[32 * j : 32 * j + NAP, nb * NB : (nb + 1) * NB],
                            lhsT,
                            feats_prev[j][:, nb * NB : (nb + 1) * NB],
                            start=first,
                            stop=stop,
                            tile_position=(0, 32 * j),
                            skip_group_check=True,
                        )

            feats_cur = [None] * 4
            for c in range(N_CHUNKS):
                for j in range(4):
                    t = 4 * c + j
                    ga, gb = 2 * t, 2 * t + 1
                    if c == 0:
                        rhs_a, rhs_b = gt0[j][:, 0, :], gt0[j][:, 1, :]
                    else:
                        gt = gts[c]
                        rhs_a, rhs_b = gt[:, 2 * j, :], gt[:, 2 * j + 1, :]
                    feat = fbuf.tile([128, B], F16, tag=f"feat{j}", name=f"feat_{t}")
                    feats_cur[j] = feat
                    for blk in range(4):
                        pre = pre_ps.tile(
                            [128, NB], F32, tag="pre", name=f"pre_{t}_{blk}"
                        )
                        # group A on PE cols 0-63, group B on cols 64-127:
                        # disjoint col-groups -> the two MMs overlap. 4 PSUM
                        # slots of one bank each keep evictions off the
                        # critical path (slot free != eviction latency).
                        col = blk * NB
                        nc.tensor.matmul(
                            pre[0:64, :],
                            wbd_sb[:, ga * 64 : (ga + 1) * 64],
                            rhs_a[:, col : col + NB],
                            start=True,
                            stop=True,
                        )
                        nc.tensor.matmul(
                            pre[64:128, :],
                            wbd_sb[:, gb * 64 : (gb + 1) * 64],
                            rhs_b[:, col : col + NB],
                            start=True,
                            stop=True,
                        )
                        dst = feat[:, col : col + NB]
                        if evict_n % 2 == 0:
                            nc.scalar.activation(
                                dst, pre[:], mybir.ActivationFunctionType.Relu
                            )
                        else:
                            nc.vector.tensor_scalar_max(dst, pre[:], 0.0)
                        evict_n += 1
                # Q head for the previous chunk's pairs (software pipeline:
                # their relu evictions finished during this chunk's stage 1).
                # Chunk 0's slot computes the raw-obs Q part instead.
                if c >= 1:
                    emit_q_pairs(c - 1, first=False, stop=False)
                else:
                    emit_obs_q()
                feats_prev, feats_cur = feats_cur, feats_prev

            emit_q_pairs(N_CHUNKS - 1, first=False, stop=True)

            nc.scalar.activation(
                qsb[:, 0 : B // 2],
                qacc[:, 0 : B // 2],
                mybir.ActivationFunctionType.Copy,
            )
            nc.vector.tensor_copy(qsb[:, B // 2 : B], qacc[:, B // 2 : B])
            nc.sync.dma_start(qp[:, 0 : B // 2], qsb[:, 0 : B // 2])
            nc.scalar.dma_start(qp[:, B // 2 : B], qsb[:, B // 2 : B])

    nc.finalize()
    return nc


def _get_program():
    global _PROGRAM
    if _PROGRAM is None:
        _PROGRAM = _build_program()
    return _PROGRAM


def _stage_inputs(observation, gvf_W, q_W, gvf_input_idxs):
    """Host-side sharding/layout. Returns in_maps (list of dicts, one per core)."""
    obs = np.asarray(observation, dtype=np.float32)
    gw = np.asarray(gvf_W, dtype=np.float32)
    qw = np.asarray(q_W, dtype=np.float32)
    idx = np.asarray(gvf_input_idxs).astype(np.int64)

    obst16 = np.ascontiguousarray(obs.T.astype(np.float16))  # (OBS_DIM, B)
    obst8 = np.ascontiguousarray(obs.T.astype(F8NP))  # (OBS_DIM, B) fp8

    in_maps = []
    for c in range(N_CORES):
        g0 = c * GPC

        # gather index plan: j = k*128 + p. Within each group the slot->dim
        # assignment is free (the block-diagonal wbd packing absorbs any
        # permutation), so sort each group's 128 slots by obs-dim: the
        # gather's descriptors then read ascending HBM addresses (better
        # bank locality on the dominant transfer block).
        k = np.arange(N_GROUPS)[:, None, None]  # group
        a = np.arange(8)[None, :, None]  # gvf within group
        i = np.arange(IPG)[None, None, :]  # input slot
        dims = idx[g0 + 8 * k + a, i].reshape(N_GROUPS, 128)  # col j0 = 16a+i
        order = np.argsort(dims, axis=1, kind="stable")  # order[k, p] = j0
        invperm = np.argsort(order, axis=1)  # invperm[k, j0] = p
        idx_flat = np.take_along_axis(dims, order, axis=1).reshape(N_GROUPS * 128)
        # wrapped for dma_gather: per call of n idxs, wrapped[p, s] = flat[s*16+p%16]
        # chunk 0 is issued as four 256-idx sub-calls, chunks 1-7 as 1024 each
        per_call = CHUNK_GROUPS * 128
        call_bounds = [(sub * 256, (sub + 1) * 256) for sub in range(4)] + [
            (chunk * per_call, (chunk + 1) * per_call) for chunk in range(1, N_CHUNKS)
        ]
        gidx_h = np.zeros((128, N_GROUPS * 8), dtype=np.int16)
        colp = 0
        for lo, hi in call_bounds:
            fl = idx_flat[lo:hi]
            ncols = (hi - lo) // 16
            wr = fl.reshape(ncols, 16).T  # (16, S): wr[p, s] = fl[s*16+p]
            gidx_h[:, colp : colp + ncols] = np.tile(wr, (8, 1))
            colp += ncols

        # block-diagonal gvf weights: wbd[p, 64k + 8a + h] = gw[g0+8k+a, h, i]
        # with p = invperm[k, 16a + i] (the sorted slot position)
        wbd_h = np.zeros((128, N_GROUPS * 64), dtype=F8NP)
        kk = np.arange(N_GROUPS)[:, None, None, None]
        aa = np.arange(8)[None, :, None, None]
        hh = np.arange(HPG)[None, None, :, None]
        ii = np.arange(IPG)[None, None, None, :]
        vals = gw[g0 + 8 * kk + aa, hh, ii]  # (64, 8, 8, 16)
        ip3 = invperm.reshape(N_GROUPS, 8, 1, IPG)  # [k, a, :, i] = p
        p_idx = np.broadcast_to(ip3, vals.shape)
        m_idx = (64 * kk + 8 * aa + hh).reshape(N_GROUPS, 8, HPG, 1)
        pf = p_idx.reshape(-1)
        mf = np.broadcast_to(m_idx, vals.shape).reshape(-1)
        wbd_h[pf, mf] = vals.astype(F8NP).reshape(-1)

        # q-head weights for gvf features: pair tile P covers feat rows
        # pp in [0,128): k = 2P + pp//64, m = pp%64, gvf = g0+8k+m//8, h = m%8
        # padded to NAP=32 action slots per pair (rows 18-31 zero)
        P = np.arange(N_PAIRS)[None, :]
        pp = np.arange(128)[:, None]
        kq = 2 * P + pp // 64
        m = pp % 64
        col = OBS_DIM + (g0 + 8 * kq + m // 8) * HPG + (m % 8)  # (128, 32)
        qwt_h = np.zeros((128, N_PAIRS, NAP), dtype=np.float16)
        qwt_h[:, :, :NA] = qw[:, col].transpose(1, 2, 0).astype(np.float16)
        qwt_h = qwt_h.reshape(128, N_PAIRS * NAP)

        # q-head weights for this core's raw-obs blocks, padded to NAP
        f0 = c * (OBS_DIM // N_CORES)
        colo = f0 + np.arange(OWN_BLKS)[None, :] * 128 + np.arange(128)[:, None]
        qwto_h = np.zeros((128, OWN_BLKS, NAP), dtype=np.float16)
        qwto_h[:, :, :NA] = qw[:, colo].transpose(1, 2, 0).astype(np.float16)
        qwto_h = qwto_h.reshape(128, OWN_BLKS * NAP)

        obso_h = np.ascontiguousarray(obst16[f0 : f0 + OWN_BLKS * 128, :])

        in_maps.append(
            {
                "obst8": obst8,
                "obso": obso_h,
                "wbd": np.ascontiguousarray(wbd_h),
                "qwt": np.ascontiguousarray(qwt_h),
                "qwto": np.ascontiguousarray(qwto_h),
                "gidx": gidx_h,
            }
        )
    return in_maps


def kernel(observation, gvf_W, q_W, gvf_input_idxs, _trace=False):
    nc = _get_program()
    in_maps = _stage_inputs(observation, gvf_W, q_W, gvf_input_idxs)
    res = run_bass_kernel_spmd(nc, in_maps, list(range(N_CORES)), trace=_trace)
    q = np.zeros((NA, B), dtype=np.float32)
    for c in range(N_CORES):
        qpc = res.results[c]["qp"].astype(np.float32)  # [128, B]
        for s in range(4):
            q += qpc[32 * s : 32 * s + NA, :]
    out = np.ascontiguousarray(q.T, dtype=np.float32)
    if _trace:
        kernel.last_exec_time_ns = res.exec_time_ns
    return out
